# revision 1
# baseline (speedup 1.0000x reference)
"""Causal self-attention block (qkv proj + 16-head causal attention + out_proj
+ c_proj) on 8 trn2 NeuronCores, data-parallel over the batch (B=8: one batch
element per core).

Layout strategy (per core, batch element b):
  - Activations are kept feature-major [feature, token] on chip so every
    linear layer is a plain   out = W_T.T @ act   matmul chain with the
    (host-pre-transposed) weight as the stationary operand. No on-device
    transposes at all.
  - Attention computes transposed scores  sT[tk, tq] = k_h.T q_h  per head
    pair (row-tiled K=64 matmuls run concurrently on the PE), exp with no
    max-subtraction (scores here are bounded by a few units), causal mask
    accumulated into the scores psum by a bf16 identity-matmul, and the AV
    product consumes sT directly with token-major V tiles as the stationary
    operand. A fused ones-row in the V operand (M=65) yields the softmax
    denominator for free; batched reciprocals and K=16 indicator matmuls
    broadcast 1/denom across partitions for the normalization.
  - All matmuls run in float32r (TF32-like, ~1e-4 rel precision, 4x the
    throughput of fp32 on the PE).
"""

import sys

if "/opt/trn_rl_repo" not in sys.path:
    sys.path.insert(0, "/opt/trn_rl_repo")

import ml_dtypes
import numpy as np

import concourse.bass as bass  # noqa: F401  (bass types used via tile/bacc)
import concourse.tile as tile
from concourse import bacc, mybir
from concourse.bass_utils import run_bass_kernel_spmd

B, T, E, H = 8, 1024, 1024, 16
DH = E // H          # 64
JQK = 2 * E          # q+k fused feature dim (2048)
F32 = mybir.dt.float32
F32R = mybir.dt.float32r
BF16 = mybir.dt.bfloat16
Act = mybir.ActivationFunctionType

TRACE = False        # test harness flips this for profiled runs
PHASE_LIMIT = 4      # debug: 1=qk proj, 2=+v, 3=+attention, 4=full
_CACHE = {}


def _emit(nc, tc, aps):
    (xT, wqkT, wvT, bqk, bvrow, woutT, bout, wcT, bc, mask01, ones,
     onesbf, outT) = aps
    ET = E // 128     # 8  e-tiles (contraction)
    TT = T // 128     # 8  token tiles
    NT = T // 512     # 2  512-wide token column chunks

    # The kernel is emitted as one software-pipelined stream so the in-order
    # PE queue always has dense matmul work while ACT paces the attention
    # exps:
    #   S1: qk features for pairs 0-3 (j-groups 0,2) + V heads 0-7
    #   S2: attention (c0+c1, pairs 0-3) interleaved with qk j-groups 1,3 +
    #       V heads 8-15
    #   S3: attention (c0, pairs 4-7); normalize c0; attention (c1, pairs
    #       4-7) interleaved with out_proj on token-chunk 0
    #   S4: normalize c1; out_proj chunk 1; c_proj
    # Pool stack (LIFO): den/y/qk/v persist; x/wqk and the S2 attention pools
    # release at the S2/S3 boundary where w3/z and the S3 attention pools
    # open in their space.
    consts = tc.alloc_tile_pool(name="consts", bufs=1)
    onesb = consts.tile([128, 128], F32R, tag="onesb")
    mask01b = consts.tile([128, 128], BF16, tag="mask01b")
    bqkb = consts.tile([128, JQK // 128], F32, tag="bqkb")
    bvb = consts.tile([1, E], F32R, tag="bvb")
    boutb = consts.tile([128, E // 128], F32, tag="boutb")
    bcb = consts.tile([128, E // 128], F32, tag="bcb")
    nc.sync.dma_start(out=onesb, in_=ones)
    nc.sync.dma_start(out=mask01b, in_=mask01)
    nc.sync.dma_start(out=bqkb, in_=bqk)
    nc.sync.dma_start(out=bvb[0:1, :], in_=bvrow)
    nc.sync.dma_start(out=boutb, in_=bout)
    nc.sync.dma_start(out=bcb, in_=bc)

    psum = tc.alloc_tile_pool(name="psum", bufs=1, space="PSUM")
    p_dram = tc.alloc_tile_pool(name="p_dram", bufs=1, space="DRAM")
    p_den = tc.alloc_tile_pool(name="p_den", bufs=1)
    p_y = tc.alloc_tile_pool(name="p_y", bufs=1)
    p_qk = tc.alloc_tile_pool(name="p_qk", bufs=1)
    p_v = tc.alloc_tile_pool(name="p_v", bufs=1)
    p_x = tc.alloc_tile_pool(name="p_x", bufs=1)
    p_wqk = tc.alloc_tile_pool(name="p_wqk", bufs=16)
    dden = p_dram.tile([64, 512], F32, tag="dden")
    denall = p_den.tile([64, 512], F32, tag="denall")
    recall = p_den.tile([64, 512], F32, tag="recall")
    yt = p_y.tile([128, ET, T], F32R)
    qkt = p_qk.tile([128, JQK // 128, T], F32R)
    vt = p_v.tile([128, TT, H, DH + 1], BF16)
    xt = p_x.tile([128, ET, T], F32R)

    def mm_psum(tag):
        return psum.tile([128, 512], F32, tag=tag, bufs=2, name="ps_" + tag)

    # ---- dense generators: qkv projection ---------------------------------
    def qk_gen(jg):
        """qkT[j, t] = Wqk x^T + bqk for the 512-wide feature group jg."""
        wtiles = []
        for et in range(ET):
            if jg == 0:                    # interleave x loads with group 0
                nc.sync.dma_start(out=xt[:, et, :],
                                  in_=xT[et * 128:(et + 1) * 128, :])
            wt = p_wqk.tile([128, 512], F32R, tag="wqk", name="wt")
            nc.sync.dma_start(out=wt, in_=wqkT[et * 128:(et + 1) * 128,
                                              jg * 512:(jg + 1) * 512])
            wtiles.append(wt)
        for js in range(4):
            jt = jg * 4 + js
            for th in range(NT):
                ps = mm_psum("mm")
                for et in range(ET):
                    nc.tensor.matmul(
                        ps,
                        wtiles[et][:, js * 128:(js + 1) * 128],
                        xt[:, et, th * 512:(th + 1) * 512],
                        start=(et == 0), stop=(et == ET - 1))
                    yield
                nc.scalar.activation(
                    out=qkt[:, jt, th * 512:(th + 1) * 512], in_=ps,
                    func=Act.Identity, bias=bqkb[:, jt:jt + 1], scale=1.0)

    def vb_gen(jh):
        """v[t, h, d] token-major for heads 8*jh..8*jh+7 (+bias via ones-row
        matmul), with a bf16 ones column at d=64 for the fused denominator."""
        if jh == 0:
            for tt in range(TT):
                nc.sync.dma_start(out=vt[:, tt, :, DH], in_=onesbf)
        wvtiles = []
        for et in range(ET):
            wt = p_wqk.tile([128, 512], F32R, tag="wqk", name="wt")
            nc.sync.dma_start(out=wt, in_=wvT[et * 128:(et + 1) * 128,
                                             jh * 512:(jh + 1) * 512])
            wvtiles.append(wt)
        for tt in range(TT):
            ps = mm_psum("mm")
            for et in range(ET):
                nc.tensor.matmul(
                    ps,
                    xt[:, et, tt * 128:(tt + 1) * 128],
                    wvtiles[et],
                    start=(et == 0), stop=False)
                yield
            nc.tensor.matmul(
                ps, onesb[0:1, 0:128], bvb[0:1, jh * 512:(jh + 1) * 512],
                start=False, stop=True)
            yield
            nc.vector.tensor_copy(
                out=vt[:, tt, jh * 8:(jh + 1) * 8, 0:DH],
                in_=ps.rearrange("p (h d) -> p h d", d=DH))

    # ---- attention generator (yields once per tk-iteration) ---------------
    LAG = 3

    def att_gen(c, a, p_esc, p_nrm):
        cs = c * 512
        last_it = 4 * c + 3
        qj = a                             # q tile of the pair
        kj = (JQK // 2) // 128 + a         # k tile of the pair
        avps = [psum.tile([128, 512], F32, tag=f"av{p}", bufs=1,
                          name=f"avp{p}") for p in range(2)]
        pend = []

        def emit_av(it, sub, clen, esc):
            for p in range(2):
                nc.tensor.matmul(
                    avps[p][0:DH + 1, sub:sub + clen],
                    vt[:, it, 2 * a + p, :],
                    esc[:, p, :clen],
                    start=(it == 0), stop=(it == last_it),
                    skip_group_check=True)

        for it in range(last_it + 1):
            n0 = it * 128
            lo = max(n0, cs)
            sub = lo - cs
            clen = 512 - sub
            scp = psum.tile([128, 2, 512], F32, tag="sc", bufs=2, name="scp")
            for p in range(2):             # paired heads: row-tiled matmuls
                pb = p * 64
                nc.tensor.matmul(
                    scp[:, p, :clen],
                    qkt[pb:pb + 64, kj, n0:n0 + 128],
                    qkt[pb:pb + 64, qj, lo:lo + clen],
                    start=True, stop=True)
            esc = p_esc.tile([128, 2, 512], BF16, tag="esc", name="esc")
            nc.scalar.activation(out=esc[:, :, :clen], in_=scp[:, :, :clen],
                                 func=Act.Exp, scale=1.0 / 8.0)
            if n0 >= cs:                   # diagonal block: causal mask,
                nc.vector.tensor_mul(      # off the PE chain thanks to LAG
                    esc[:, :, 0:128], esc[:, :, 0:128],
                    mask01b[:, None, :].broadcast_to([128, 2, 128]))
            pend.append((it, sub, clen, esc))
            if len(pend) > LAG:
                emit_av(*pend.pop(0))
            yield
        for args in pend:
            emit_av(*args)
        for p in range(2):                 # drain unnormalized y + denom row
            h = 2 * a + p
            nc.vector.tensor_copy(out=yt[p * 64:p * 64 + 64, qj,
                                         cs:cs + 512],
                                  in_=avps[p][0:DH, :])
            # engines can only address partition bases that are multiples of
            # 32, so stage the denominator row at partition 64 and DMA-
            # scatter it (partition-agnostic) into denall's row.
            stg = p_nrm.tile([128, 512], F32, tag="stg", bufs=2, name="stg")
            nc.vector.tensor_copy(out=stg[64:65, :],
                                  in_=avps[p][DH:DH + 1, :])
            nc.sync.dma_start(out=denall[32 * c + h:32 * c + h + 1, :],
                              in_=stg[64:65, :])

    def norm_unit(c, p_nrm):
        """Batched 1/denom for chunk c, bounced through DRAM and partition-
        broadcast back; normalizes y in place on DVE."""
        r0 = 32 * c
        cs = c * 512
        with nc.allow_low_precision(reason="fp32 reciprocal feeding an f32r "
                                    "multiply; well inside tolerance"):
            nc.vector.reciprocal(out=recall[r0:r0 + 16, :],
                                 in_=denall[r0:r0 + 16, :])
        nc.sync.dma_start(out=dden[r0:r0 + 16, :], in_=recall[r0:r0 + 16, :])
        for a in range(H // 2):
            rb = p_nrm.tile([128, 512], F32, tag="rb", bufs=2, name="rb")
            for p in range(2):
                row = dden[r0 + 2 * a + p:r0 + 2 * a + p + 1, :]
                src = bass.AP(tensor=row.tensor, offset=row.offset,
                              ap=[[0, 64]] + list(row.ap)[1:])
                nc.sync.dma_start(out=rb[p * 64:(p + 1) * 64, :], in_=src)
            nc.vector.tensor_mul(yt[:, a, cs:cs + 512],
                                 yt[:, a, cs:cs + 512], rb)

    # ---- drivers ----------------------------------------------------------
    def run_dense(dense, n=None):
        steps = 0
        while dense and (n is None or steps < n):
            try:
                next(dense[0])
                steps += 1
            except StopIteration:
                dense.pop(0)
        return steps

    def drive(att_units, dense, ratio=5):
        att_units = list(att_units)
        while att_units:
            try:
                next(att_units[0])
            except StopIteration:
                att_units.pop(0)
                continue
            run_dense(dense, ratio)
        run_dense(dense)

    # S1: dense deps for attention pairs 0-3
    dense1 = [qk_gen(0), qk_gen(2)] + ([vb_gen(0)] if PHASE_LIMIT >= 2 else [])
    run_dense(dense1)

    # S2: attention pairs 0-3 (both chunks) over the remaining qkv work
    p_esc1 = tc.alloc_tile_pool(name="p_esc1", bufs=4)
    p_nrm1 = tc.alloc_tile_pool(name="p_nrm1", bufs=1)
    dense2 = [qk_gen(1), qk_gen(3)] + ([vb_gen(1)] if PHASE_LIMIT >= 2 else [])
    att2 = [att_gen(c, a, p_esc1, p_nrm1)
            for a in range(4) for c in range(NT)] if PHASE_LIMIT >= 3 else []
    drive(att2, dense2)
    p_nrm1.release()
    p_esc1.release()
    p_wqk.release()
    p_x.release()

    # S3: remaining attention; out_proj weight prefetch + chunk-0 out_proj
    p_w3 = tc.alloc_tile_pool(name="p_w3", bufs=16)
    p_z = tc.alloc_tile_pool(name="p_z", bufs=1)
    p_esc2 = tc.alloc_tile_pool(name="p_esc2", bufs=4)
    p_nrm2 = tc.alloc_tile_pool(name="p_nrm2", bufs=1)
    zt = p_z.tile([128, ET, T], F32R)
    wout_tiles = []
    if PHASE_LIMIT >= 4:
        for og in range(2):
            for et in range(ET):
                wt = p_w3.tile([128, 512], F32R, tag="w3", name="wt3")
                nc.sync.dma_start(
                    out=wt, in_=woutT[et * 128:(et + 1) * 128,
                                      og * 512:(og + 1) * 512])
                wout_tiles.append(wt)

    def oproj_gen(th):
        for og in range(2):
            for os_ in range(4):
                ot = og * 4 + os_
                ps = mm_psum("mm")
                for et in range(ET):
                    nc.tensor.matmul(
                        ps,
                        wout_tiles[og * ET + et][:, os_ * 128:(os_ + 1) * 128],
                        yt[:, et, th * 512:(th + 1) * 512],
                        start=(et == 0), stop=(et == ET - 1))
                    yield
                nc.scalar.activation(
                    out=zt[:, ot, th * 512:(th + 1) * 512], in_=ps,
                    func=Act.Identity, bias=boutb[:, ot:ot + 1], scale=1.0)

    def cproj_gen(wts, og, th):
        for os_ in range(4):
            ot = og * 4 + os_
            ps = mm_psum("mm")
            for et in range(ET):
                nc.tensor.matmul(
                    ps,
                    wts[et][:, os_ * 128:(os_ + 1) * 128],
                    zt[:, et, th * 512:(th + 1) * 512],
                    start=(et == 0), stop=(et == ET - 1))
                yield
            ob = p_out.tile([128, 512], F32, tag="ob", name="ob")
            nc.scalar.activation(out=ob, in_=ps, func=Act.Identity,
                                 bias=bcb[:, ot:ot + 1], scale=1.0)
            nc.sync.dma_start(
                out=outT[ot * 128:(ot + 1) * 128, th * 512:(th + 1) * 512],
                in_=ob)

    if PHASE_LIMIT >= 3:
        drive([att_gen(0, a, p_esc2, p_nrm2) for a in range(4, 8)], [])
        norm_unit(0, p_nrm2)
        drive([att_gen(1, a, p_esc2, p_nrm2) for a in range(4, 8)],
              [oproj_gen(0)] if PHASE_LIMIT >= 4 else [])
        norm_unit(1, p_nrm2)
    p_nrm2.release()
    p_esc2.release()

    # S4: c_proj — its own weight pool so zt-chunk-0 c_proj can run ahead of
    # oproj(1) (which waits on the chunk-1 normalization chain).
    p_wc = tc.alloc_tile_pool(name="p_wc", bufs=9)
    p_out = tc.alloc_tile_pool(name="p_out", bufs=2)

    def load_wc(og):
        wts = []
        for et in range(ET):
            wt = p_wc.tile([128, 512], F32R, tag="wc", name="wtc")
            nc.sync.dma_start(out=wt, in_=wcT[et * 128:(et + 1) * 128,
                                             og * 512:(og + 1) * 512])
            wts.append(wt)
        return wts

    if PHASE_LIMIT >= 4:
        wc0 = load_wc(0)
        run_dense([cproj_gen(wc0, 0, 0)])
        run_dense([oproj_gen(1)])
        wc1 = load_wc(1)
        run_dense([cproj_gen(wc0, 0, 1)])
        run_dense([cproj_gen(wc1, 1, 0), cproj_gen(wc1, 1, 1)])
    p_out.release()
    p_wc.release()
    p_z.release()
    p_w3.release()
    p_v.release()
    p_qk.release()
    p_y.release()
    p_den.release()
    p_dram.release()
    psum.release()
    consts.release()


def _build():
    if "nc" in _CACHE:
        return _CACHE["nc"]
    nc = bacc.Bacc("TRN2", target_bir_lowering=False, debug=False,
                   enable_asserts=True, num_devices=8)
    d = nc.dram_tensor
    aps = [
        d("xT", [E, T], F32R, kind="ExternalInput").ap(),
        d("wqkT", [E, JQK], F32R, kind="ExternalInput").ap(),
        d("wvT", [E, E], F32R, kind="ExternalInput").ap(),
        d("bqk", [128, JQK // 128], F32, kind="ExternalInput").ap(),
        d("bvrow", [1, E], F32R, kind="ExternalInput").ap(),
        d("woutT", [E, E], F32R, kind="ExternalInput").ap(),
        d("bout", [128, E // 128], F32, kind="ExternalInput").ap(),
        d("wcT", [E, E], F32R, kind="ExternalInput").ap(),
        d("bc", [128, E // 128], F32, kind="ExternalInput").ap(),
        d("mask01", [128, 128], BF16, kind="ExternalInput").ap(),
        d("ones", [128, 128], F32R, kind="ExternalInput").ap(),
        d("onesbf", [128, H], BF16, kind="ExternalInput").ap(),
        d("outT", [E, T], F32, kind="ExternalOutput").ap(),
    ]
    with tile.TileContext(nc) as tc:
        _emit(nc, tc, aps)
    nc.compile()
    _CACHE["nc"] = nc
    return nc


def _host_inputs(x, in_proj_w, in_proj_b, out_proj_w, out_proj_b,
                 c_proj_w, c_proj_b):
    f = np.float32
    x = np.ascontiguousarray(np.asarray(x, f))
    in_proj_w = np.asarray(in_proj_w, f)
    in_proj_b = np.asarray(in_proj_b, f)
    shared = {
        "wqkT": np.ascontiguousarray(in_proj_w[:JQK].T),
        "wvT": np.ascontiguousarray(in_proj_w[JQK:].T),
        "bqk": np.ascontiguousarray(in_proj_b[:JQK].reshape(JQK // 128, 128).T),
        "bvrow": np.ascontiguousarray(in_proj_b[JQK:].reshape(1, E)),
        "woutT": np.ascontiguousarray(np.asarray(out_proj_w, f).T),
        "bout": np.ascontiguousarray(
            np.asarray(out_proj_b, f).reshape(E // 128, 128).T),
        "wcT": np.ascontiguousarray(np.asarray(c_proj_w, f).T),
        "bc": np.ascontiguousarray(
            np.asarray(c_proj_b, f).reshape(E // 128, 128).T),
        "mask01": np.where(np.arange(128)[None, :] >= np.arange(128)[:, None],
                           f(1.0), f(0.0)).astype(ml_dtypes.bfloat16),
        "ones": np.ones((128, 128), f),
        "onesbf": np.ones((128, H), ml_dtypes.bfloat16),
    }
    return [{**shared, "xT": np.ascontiguousarray(x[b].T)} for b in range(B)]


def kernel(x, in_proj_w, in_proj_b, out_proj_w, out_proj_b, c_proj_w,
           c_proj_b):
    nc = _build()
    in_maps = _host_inputs(x, in_proj_w, in_proj_b, out_proj_w, out_proj_b,
                           c_proj_w, c_proj_b)
    res = run_bass_kernel_spmd(nc, in_maps, core_ids=list(range(B)),
                               trace=TRACE)
    _CACHE["last_result"] = res
    out = np.stack([res.results[b]["outT"].T for b in range(B)])
    return np.ascontiguousarray(out, dtype=np.float32)



# revision 3
# speedup vs baseline: 1.1363x; 1.1363x over previous
"""Causal self-attention block (qkv proj + 16-head causal attention + out_proj
+ c_proj) on 8 trn2 NeuronCores, data-parallel over the batch (B=8: one batch
element per core).

v2 schedule: one continuous software-pipelined stream built around the PE.

  - Attention runs chunk-major over query chunks (512, 256, 256) with all 16
    heads (8 row-tiled pairs) per chunk; the qkv / out_proj / c_proj matmuls
    form a dense filler stream pumped between attention steps so the PE never
    idles (keeping it at the 2.4 GHz p-state). A marker/gate system keeps
    emission order compatible with the in-order engine queues.
  - The causal mask is folded into the scores psum with one extra
    (maskT @ [I|I]) matmul on diagonal blocks - no post-exp masking.
  - Softmax denominators come free from a fused ones-column in the AV
    stationary operand; rows are gathered per chunk into a [16, T] table via
    tiny DMAs, inverted with reciprocal_approx_fast, and partition-broadcast
    back with K=16 indicator matmuls (no DRAM bounce, no broadcast DMA fan).
  - qkv/attention/c_proj paths run in bf16 (psum accumulation in fp32);
    out_proj stays f32r. The PE is warmed with dummy matmuls while the first
    DMAs land.
  - DMA queues: sync = x + all weights; scalar = activations only; gpsimd =
    consts/memset/output stores; vector = denominator row gathers.
"""

import sys

if "/opt/trn_rl_repo" not in sys.path:
    sys.path.insert(0, "/opt/trn_rl_repo")

import ml_dtypes
import numpy as np

import concourse.bass as bass  # noqa: F401
import concourse.tile as tile
from concourse import bacc, mybir
from concourse.bass_utils import run_bass_kernel_spmd

B, T, E, H = 8, 1024, 1024, 16
DH = E // H          # 64
JQK = 2 * E          # 2048
ET = E // 128        # 8
TT = T // 128        # 8
F32 = mybir.dt.float32
F32R = mybir.dt.float32r
BF16 = mybir.dt.bfloat16
Act = mybir.ActivationFunctionType

CHUNKS = [(0, 512), (512, 256), (768, 256)]   # (col start, width)

TRACE = False        # test harness flips this for profiled runs
_CACHE = {}


def _emit(nc, tc, aps):
    (xT, wqkT, wvT, bqk, bvrow, woutT, bout, wcT, bc,
     maskT, id2, onesrow, Ea, outT) = aps

    # ---- pools (alloc order = LIFO release stack) --------------------------
    consts = tc.alloc_tile_pool(name="consts", bufs=1)
    maskTb = consts.tile([128, 128], BF16, tag="maskTb")
    id2b = consts.tile([128, 2, 128], BF16, tag="id2b")
    onesrowb = consts.tile([1, 128], BF16, tag="onesrowb")
    Eab = consts.tile([16, 8, 128], BF16, tag="Eab")
    bqkb = consts.tile([128, 16], F32, tag="bqkb")
    bvb = consts.tile([1, E], BF16, tag="bvb")
    boutb = consts.tile([128, 8], F32, tag="boutb")
    bcb = consts.tile([128, 8], F32, tag="bcb")
    for dst, src in ((maskTb, maskT), (id2b, id2), (onesrowb, onesrow),
                     (Eab, Ea), (bqkb, bqk), (bvb, bvrow), (boutb, bout),
                     (bcb, bc)):
        nc.gpsimd.dma_start(out=dst, in_=src)

    psum = tc.alloc_tile_pool(name="psum", bufs=1, space="PSUM")
    p_den = tc.alloc_tile_pool(name="p_den", bufs=1)
    p_qk = tc.alloc_tile_pool(name="p_qk", bufs=1)
    p_v = tc.alloc_tile_pool(name="p_v", bufs=1)
    p_y = tc.alloc_tile_pool(name="p_y", bufs=1)
    p_esc = tc.alloc_tile_pool(name="p_esc", bufs=4)
    p_stg = tc.alloc_tile_pool(name="p_stg", bufs=2)
    p_ob = tc.alloc_tile_pool(name="p_ob", bufs=2)
    p_wout = tc.alloc_tile_pool(name="p_wout", bufs=16)
    p_x = tc.alloc_tile_pool(name="p_x", bufs=1)
    p_w = tc.alloc_tile_pool(name="p_w", bufs=24)

    denall = p_den.tile([16, T], F32, tag="denall")
    recf = p_den.tile([16, T], F32, tag="recf")
    recbf = p_den.tile([16, T], BF16, tag="recbf")
    qkt = p_qk.tile([128, 16, T], BF16)
    vt = p_v.tile([128, TT, H, DH + 1], BF16)
    yt = p_y.tile([128, ET, T], F32R)
    xt = p_x.tile([128, ET, T], BF16)
    zt = None          # allocated after the x/w pools release

    def mm_psum():
        return psum.tile([128, 512], F32, tag="mm", bufs=2, name="ps_mm")

    # ---- driver with emission-order gates ----------------------------------
    done = set()

    def pump(dense, n):
        steps = 0
        while dense and steps < n:
            try:
                next(dense[0])
                steps += 1
            except StopIteration:
                dense.pop(0)
        return steps

    def pump_all(dense):
        while dense:
            try:
                next(dense[0])
            except StopIteration:
                dense.pop(0)

    def drive(units, dense, ratio):
        for needs, gen in units:
            while not all(m in done for m in needs):
                if pump(dense, 1) == 0:
                    raise RuntimeError(f"dense exhausted before {needs}")
            for _ in gen:
                pump(dense, ratio)

    # ---- dense generators ---------------------------------------------------
    def warm_gen(n=14):
        wps = mm_psum()
        for _ in range(n):
            nc.tensor.matmul(wps[:, 0:128], id2b[:, 0, :], id2b[:, 0, :],
                             start=True, stop=True, skip_group_check=True)
            yield

    wqk_tiles = {}

    def load_wqk(jg):
        ts = []
        for et in range(ET):
            wt = p_w.tile([128, 512], BF16, tag="w", name="wt")
            nc.sync.dma_start(out=wt, in_=wqkT[et * 128:(et + 1) * 128,
                                              jg * 512:(jg + 1) * 512])
            ts.append(wt)
        wqk_tiles[jg] = ts

    def wload_gen(jgs):
        for jg in jgs:
            load_wqk(jg)
            yield

    def qk_step(a):
        """q j-tile a + k j-tile 8+a over both token halves, th-major so the
        tokens 0-511 parts land first (chunk-0 attention gates on them)."""
        js = a % 4
        jgs = (0, 2) if a < 4 else (1, 3)
        for th in range(2):
            for jg in jgs:
                jt = jg * 4 + js
                ps = mm_psum()
                for et in range(ET):
                    nc.tensor.matmul(
                        ps,
                        wqk_tiles[jg][et][:, js * 128:(js + 1) * 128],
                        xt[:, et, th * 512:(th + 1) * 512],
                        start=(et == 0), stop=(et == ET - 1))
                    yield
                nc.scalar.activation(
                    out=qkt[:, jt, th * 512:(th + 1) * 512], in_=ps,
                    func=Act.Identity, bias=bqkb[:, jt:jt + 1], scale=1.0)
        done.add(f"qk{a}")

    def vb_gen(jh):
        """v[t, h, d] token-major for heads 8jh..8jh+7 (+bias via a ones-row
        matmul); column DH of vt is a ones block (memset) for the fused
        softmax denominator."""
        wv_tiles = []
        for et in range(ET):
            wt = p_w.tile([128, 512], BF16, tag="w", name="wtv")
            nc.sync.dma_start(out=wt, in_=wvT[et * 128:(et + 1) * 128,
                                             jh * 512:(jh + 1) * 512])
            wv_tiles.append(wt)
        if jh == 0:
            nc.gpsimd.memset(vt[:, :, :, DH], 1.0)
        for tt in range(TT):
            ps = mm_psum()
            for et in range(ET):
                nc.tensor.matmul(
                    ps,
                    xt[:, et, tt * 128:(tt + 1) * 128],
                    wv_tiles[et],
                    start=(et == 0), stop=False)
                yield
            nc.tensor.matmul(
                ps, onesrowb, bvb[0:1, jh * 512:(jh + 1) * 512],
                start=False, stop=True)
            yield
            nc.vector.tensor_copy(
                out=vt[:, tt, jh * 8:(jh + 1) * 8, 0:DH],
                in_=ps.rearrange("p (h d) -> p h d", d=DH))
            if tt == 3:
                done.add(f"vb{jh}a")
        done.add(f"vb{jh}b")

    # ---- attention ----------------------------------------------------------
    LAG = 3

    def att_gen(ci, a):
        cs, cw = CHUNKS[ci]
        last_it = (cs + cw) // 128 - 1
        kj = 8 + a
        avps = [psum.tile([128, 512], F32, tag=f"av{p}", bufs=1,
                          name=f"avp{p}") for p in range(2)]
        pend = []

        def emit_av(it, sub, clen, esc):
            for p in range(2):
                nc.tensor.matmul(
                    avps[p][0:DH + 1, sub:sub + clen],
                    vt[:, it, 2 * a + p, :],
                    esc[:, p, :clen],
                    start=(it == 0), stop=(it == last_it),
                    skip_group_check=True)

        for it in range(last_it + 1):
            n0 = it * 128
            lo = max(n0, cs)
            sub = lo - cs
            clen = cw - sub
            diag = n0 >= cs
            scp = psum.tile([128, 2, 512], F32, tag="sc", bufs=2, name="scp")
            for p in range(2):             # row-tiled pair: runs concurrently
                pb = p * 64
                nc.tensor.matmul(
                    scp[:, p, :clen],
                    qkt[pb:pb + 64, kj, n0:n0 + 128],
                    qkt[pb:pb + 64, a, lo:lo + clen],
                    start=True, stop=(not diag), skip_group_check=True)
            if diag:                       # causal mask folded into the psum
                nc.tensor.matmul(
                    scp[:, :, 0:128], maskTb, id2b,
                    start=False, stop=True, skip_group_check=True)
            esc = p_esc.tile([128, 2, 512], BF16, tag="esc", name="esc")
            nc.scalar.activation(out=esc[:, :, :clen], in_=scp[:, :, :clen],
                                 func=Act.Exp, scale=1.0 / 8.0)
            pend.append((it, sub, clen, esc))
            if len(pend) > LAG:
                emit_av(*pend.pop(0))
            yield
        while pend:
            emit_av(*pend.pop(0))
            yield
        for p in range(2):                 # drain y + denominator row
            h = 2 * a + p
            nc.vector.tensor_copy(out=yt[p * 64:p * 64 + 64, a, cs:cs + cw],
                                  in_=avps[p][0:DH, :cw])
            stg = p_stg.tile([128, 512], F32, tag="stg", bufs=2, name="stg")
            nc.vector.tensor_copy(out=stg[64:65, :cw],
                                  in_=avps[p][DH:DH + 1, :cw])
            nc.gpsimd.dma_start(out=denall[h:h + 1, cs:cs + cw],
                                in_=stg[64:65, :cw])

    def norm_gen(ci):
        """1/denominator for chunk ci, partition-broadcast via K=16 indicator
        matmuls; normalizes yt in place."""
        cs, cw = CHUNKS[ci]
        with nc.allow_low_precision(reason="softmax reciprocal (18-bit ok)"):
            nc.vector.reciprocal_approx_fast(out=recf[:, cs:cs + cw],
                                             in_=denall[:, cs:cs + cw])
            nc.vector.tensor_copy(out=recbf[:, cs:cs + cw],
                                  in_=recf[:, cs:cs + cw])
        yield
        yield
        for a in range(8):
            rb = psum.tile([128, 512], F32, tag="mm", bufs=2, name="rb")
            nc.tensor.matmul(rb[:, :cw], Eab[:, a, :], recbf[0:16, cs:cs + cw],
                             start=True, stop=True)
            yield
            nc.vector.tensor_mul(yt[:, a, cs:cs + cw], yt[:, a, cs:cs + cw],
                                 rb[:, :cw])
            yield
        done.add(f"norm{ci}")

    # ---- output projections -------------------------------------------------
    wout_tiles = []

    def wout_load_gen():
        for og in range(2):
            for et in range(ET):
                wt = p_wout.tile([128, 512], F32R, tag="wo", name="wo")
                nc.sync.dma_start(out=wt, in_=woutT[et * 128:(et + 1) * 128,
                                                    og * 512:(og + 1) * 512])
                wout_tiles.append(wt)
            yield

    wc_tiles = []

    def wc_load_gen():
        for og in range(2):
            for et in range(ET):
                wt = p_wc.tile([128, 512], BF16, tag="wc", name="wc")
                nc.sync.dma_start(out=wt, in_=wcT[et * 128:(et + 1) * 128,
                                                 og * 512:(og + 1) * 512])
                wc_tiles.append(wt)
            yield

    def oproj_gen(ci):
        cs, cw = CHUNKS[ci]
        for og in range(2):
            for os_ in range(4):
                ot = og * 4 + os_
                ps = mm_psum()
                for et in range(ET):
                    nc.tensor.matmul(
                        ps[:, :cw],
                        wout_tiles[og * ET + et][:, os_ * 128:(os_ + 1) * 128],
                        yt[:, et, cs:cs + cw],
                        start=(et == 0), stop=(et == ET - 1))
                    yield
                nc.scalar.activation(
                    out=zt[:, ot, cs:cs + cw], in_=ps[:, :cw],
                    func=Act.Identity, bias=boutb[:, ot:ot + 1], scale=1.0)

    def cproj_gen(og, ci):
        cs, cw = CHUNKS[ci]
        for os_ in range(4):
            ot = og * 4 + os_
            ps = mm_psum()
            for et in range(ET):
                nc.tensor.matmul(
                    ps[:, :cw],
                    wc_tiles[og * ET + et][:, os_ * 128:(os_ + 1) * 128],
                    zt[:, et, cs:cs + cw],
                    start=(et == 0), stop=(et == ET - 1))
                yield
            ob = p_ob.tile([128, 512], F32, tag="ob", name="ob")
            nc.vector.tensor_scalar_add(out=ob[:, :cw], in0=ps[:, :cw],
                                        scalar1=bcb[:, ot:ot + 1])
            nc.gpsimd.dma_start(
                out=outT[ot * 128:(ot + 1) * 128, cs:cs + cw],
                in_=ob[:, :cw])

    # ---- master schedule ----------------------------------------------------
    # startup: x halves (tokens 0-511 first) + first weight groups, then warm
    # the PE p-state while they land.
    for et in range(ET):
        nc.sync.dma_start(out=xt[:, et, 0:512],
                          in_=xT[et * 128:(et + 1) * 128, 0:512])
    for et in range(ET):
        nc.sync.dma_start(out=xt[:, et, 512:1024],
                          in_=xT[et * 128:(et + 1) * 128, 512:1024])
    load_wqk(0)
    load_wqk(2)
    pump_all([warm_gen(), qk_step(0), vb_gen(0)])

    dense = [qk_step(1), qk_step(2), qk_step(3), wload_gen([1, 3]),
             qk_step(4), qk_step(5), qk_step(6), qk_step(7), vb_gen(1),
             wout_load_gen()]
    units_c0 = [((f"qk{a}", "vb0a" if a < 4 else "vb1a"), att_gen(0, a))
                for a in range(8)]
    drive(units_c0, dense, ratio=8)

    p_w.release()
    p_x.release()
    p_z = tc.alloc_tile_pool(name="p_z", bufs=1)
    p_wc = tc.alloc_tile_pool(name="p_wc", bufs=16)
    zt = p_z.tile([128, ET, T], BF16)

    dense += [norm_gen(0), wc_load_gen(), oproj_gen(0),
              cproj_gen(0, 0), cproj_gen(1, 0)]
    units_c1 = [(("vb0b" if a < 4 else "vb1b",), att_gen(1, a))
                for a in range(8)]
    drive(units_c1, dense, ratio=6)

    dense += [norm_gen(1), oproj_gen(1), cproj_gen(0, 1), cproj_gen(1, 1)]
    units_c2 = [((), att_gen(2, a)) for a in range(8)]
    drive(units_c2, dense, ratio=6)

    dense += [norm_gen(2), oproj_gen(2), cproj_gen(0, 2), cproj_gen(1, 2)]
    pump_all(dense)

    p_wc.release()
    p_z.release()
    p_wout.release()
    p_ob.release()
    p_stg.release()
    p_esc.release()
    p_y.release()
    p_v.release()
    p_qk.release()
    p_den.release()
    psum.release()
    consts.release()


def _build():
    if "nc" in _CACHE:
        return _CACHE["nc"]
    nc = bacc.Bacc("TRN2", target_bir_lowering=False, debug=False,
                   enable_asserts=True, num_devices=8)
    d = nc.dram_tensor
    aps = [
        d("xT", [E, T], BF16, kind="ExternalInput").ap(),
        d("wqkT", [E, JQK], BF16, kind="ExternalInput").ap(),
        d("wvT", [E, E], BF16, kind="ExternalInput").ap(),
        d("bqk", [128, 16], F32, kind="ExternalInput").ap(),
        d("bvrow", [1, E], BF16, kind="ExternalInput").ap(),
        d("woutT", [E, E], F32R, kind="ExternalInput").ap(),
        d("bout", [128, 8], F32, kind="ExternalInput").ap(),
        d("wcT", [E, E], BF16, kind="ExternalInput").ap(),
        d("bc", [128, 8], F32, kind="ExternalInput").ap(),
        d("maskT", [128, 128], BF16, kind="ExternalInput").ap(),
        d("id2", [128, 2, 128], BF16, kind="ExternalInput").ap(),
        d("onesrow", [1, 128], BF16, kind="ExternalInput").ap(),
        d("Ea", [16, 8, 128], BF16, kind="ExternalInput").ap(),
        d("outT", [E, T], F32, kind="ExternalOutput").ap(),
    ]
    with tile.TileContext(nc) as tc:
        _emit(nc, tc, aps)
    nc.compile()
    _CACHE["nc"] = nc
    return nc


def _host_inputs(x, in_proj_w, in_proj_b, out_proj_w, out_proj_b,
                 c_proj_w, c_proj_b):
    f = np.float32
    bf = ml_dtypes.bfloat16
    x = np.ascontiguousarray(np.asarray(x, f))
    in_proj_w = np.asarray(in_proj_w, f)
    in_proj_b = np.asarray(in_proj_b, f)
    r = np.arange(128)
    id2 = np.zeros((128, 2, 128), f)
    id2[r, 0, r] = 1.0
    id2[r, 1, r] = 1.0
    Ea = np.zeros((16, 8, 128), f)
    for a in range(8):
        Ea[2 * a, a, 0:64] = 1.0
        Ea[2 * a + 1, a, 64:128] = 1.0
    shared = {
        "wqkT": np.ascontiguousarray(in_proj_w[:JQK].T).astype(bf),
        "wvT": np.ascontiguousarray(in_proj_w[JQK:].T).astype(bf),
        "bqk": np.ascontiguousarray(in_proj_b[:JQK].reshape(16, 128).T),
        "bvrow": in_proj_b[JQK:].reshape(1, E).astype(bf),
        "woutT": np.ascontiguousarray(np.asarray(out_proj_w, f).T),
        "bout": np.ascontiguousarray(
            np.asarray(out_proj_b, f).reshape(8, 128).T),
        "wcT": np.ascontiguousarray(np.asarray(c_proj_w, f).T).astype(bf),
        "bc": np.ascontiguousarray(
            np.asarray(c_proj_b, f).reshape(8, 128).T),
        "maskT": np.where(r[None, :] > r[:, None], f(-200.0),
                          f(0.0)).astype(bf),
        "id2": id2.astype(bf),
        "onesrow": np.ones((1, 128), bf),
        "Ea": Ea.astype(bf),
    }
    return [{**shared, "xT": np.ascontiguousarray(x[b].T).astype(bf)}
            for b in range(B)]


def kernel(x, in_proj_w, in_proj_b, out_proj_w, out_proj_b, c_proj_w,
           c_proj_b):
    nc = _build()
    in_maps = _host_inputs(x, in_proj_w, in_proj_b, out_proj_w, out_proj_b,
                           c_proj_w, c_proj_b)
    res = run_bass_kernel_spmd(nc, in_maps, core_ids=list(range(B)),
                               trace=TRACE)
    _CACHE["last_result"] = res
    out = np.stack([res.results[b]["outT"].T for b in range(B)])
    return np.ascontiguousarray(out, dtype=np.float32)
